# revision 1
# baseline (speedup 1.0000x reference)
"""CRF loss (forward-algorithm partition function) on 8 Trainium2 cores.

Strategy
--------
Batch (B=64) is sharded 8 ways -> 8 sequences per core.  The lax.scan
over L=512 steps is computed in *linear* space: with

    E_l = exp(scores_l - C),   C = log(T) + 0.5

the log-space recurrence  p_{l}[t'] = logsumexp_t(scores_l[t,t'] + p_{l-1}[t])
becomes  w_l = E_l^T w_{l-1},  with  p_l = log(w_l) + s0 + l*C  recovered at
the end (drift of log|w| stays within +-1 for N(0,1) scores, so fp32 is safe
-- validated to ~2.5e-6 absolute partition error).

Per core the 511-step chain is run as tiny TensorE matvecs: the exp'd score
tile for two batch rows is packed [128=(2b x 64t), 64=t'] and used as the
*stationary* operand (lhsT); the running vector w is the N=1 moving operand;
the output column lands in PSUM and one [128,8] DVE copy per step moves all
8 batch rows' new vectors back to SBUF.  exp() is done by ScalarE on big
[128, 32*64] tiles, off the critical path.

The tiny remainder (gold-path gather, softmax weight, final log/sum) is done
on the host -- it touches 0.02% of the data.
"""

import os
import threading
import numpy as np

L, B, T = 512, 64, 64
NCORES = 8
B_LOC = B // NCORES            # 8 sequences per core
NPAIR = B_LOC // 2             # 4 partition-pairs per core
NSTEP = L - 1                  # 511 chain steps (l = 1..511)
KB = 32                        # chain steps exp'd/DMA'd per block
C_SHIFT = float(np.log(T) + 0.5)
START_TAG = 0
END_TAG = 1

_nc_cache = [None]
_nc_lock = threading.Lock()
LAST_RESULTS = [None]          # test.py reads exec_time_ns from here


def _enable_ldw_opt():
    """Flip walrus's --enable-ldw-opt to true: consecutive matmuls that
    share a stationary operand (our per-pair MM1/MM2) then skip the
    redundant LDWEIGHTS."""
    import concourse.bass_utils as bu

    if getattr(bu.run_command, "_ldw_patched", False):
        return
    orig = bu.run_command

    def patched(cmd, *a, **kw):
        cmd = [
            c.replace("--enable-ldw-opt=false", "--enable-ldw-opt=true")
            if isinstance(c, str)
            else c
            for c in cmd
        ]
        return orig(cmd, *a, **kw)

    patched._ldw_patched = True
    bu.run_command = patched


def _build_nc():
    import concourse.bacc as bacc
    import concourse.mybir as mybir
    import concourse.tile as tile

    # note: walrus --enable-ldw-opt=true hard-rejects the standalone
    # InstLdweights that bacc's move_matmul_waits_to_ldweights emits, so
    # this stays off unless explicitly requested for experiments
    if bool(int(os.environ.get("KERNEL_LDW_OPT", "0"))):
        _enable_ldw_opt()

    dt = mybir.dt
    nc = bacc.Bacc("TRN2", target_bir_lowering=False, debug=False)

    scores_d = nc.declare_dram_parameter(
        "scores_loc", [L, B_LOC, T, T], dt.float32, isOutput=False
    )
    rhs_init_d = nc.declare_dram_parameter(
        "rhs_init", [128, 16], dt.float32, isOutput=False
    )
    out_d = nc.declare_dram_parameter("w_out", [128, 8], dt.float32, isOutput=True)

    blocks = []
    l0 = 1
    while l0 < L:
        nst = min(KB, L - l0)
        blocks.append((l0, nst))
        l0 += nst

    with tile.TileContext(nc) as tc:
        with (
            tc.tile_pool(name="raw", bufs=2) as raw_pool,
            tc.tile_pool(name="exp", bufs=2) as exp_pool,
            tc.tile_pool(name="state", bufs=1) as state_pool,
            tc.tile_pool(name="psum", bufs=1, space="PSUM") as psum_pool,
        ):
            rhs = state_pool.tile([128, 16], dt.bfloat16)
            rhs_stage = state_pool.tile([128, 16], dt.float32)
            zeros = state_pool.tile([128, 16], dt.float32)
            out_stage = state_pool.tile([128, 8], dt.float32)
            # one PSUM tile (= one bank) per half-group so group B's
            # matmul writes don't serialize against group A's DVE read
            # (same-bank PE-W + DVE-R is serialized by the hardware)
            psums = [
                psum_pool.tile([128, 8], dt.float32, name=f"psum_g{g}")
                for g in range(2)
            ]

            nc.sync.dma_start(rhs_stage[:], rhs_init_d[:])
            nc.vector.tensor_copy(rhs[:], rhs_stage[:])  # fp32 -> bf16
            nc.vector.memset(zeros[:], 0.0)
            # Pre-zero PSUM once: matvec outputs only ever write the
            # [0:64, even-col] / [64:128, odd-col] windows, so the
            # complementary windows stay exactly 0 forever and the per-step
            # copy propagates those zeros into the rhs zero slots.
            for g in range(2):
                nc.vector.tensor_copy(psums[g][:], zeros[:, 0:8])

            step = 0
            for (l0, nst) in blocks:
                tiles = []
                for q in range(NPAIR):
                    t_raw = raw_pool.tile([128, nst * T], dt.float32, tag=f"raw{q}")
                    t = exp_pool.tile([128, nst * T], dt.bfloat16, tag=f"pair{q}")
                    src = scores_d[l0 : l0 + nst, 2 * q : 2 * q + 2, :, :].rearrange(
                        "j b t u -> (b t) j u"
                    )
                    dst = t_raw[:].rearrange("p (j u) -> p j u", u=T)
                    # alternate HWDGE (sync) and SWDGE (gpsimd) so the two
                    # 1MB streams overlap on different DMA queues
                    dma_eng = nc.sync if q % 2 == 0 else nc.gpsimd
                    dma_eng.dma_start(dst, src)
                    # bf16 exp output: single-pass LDWEIGHTS/MATMUL on the PE
                    # (fp32 would run in double-pass LOW_HIGH mode).  The
                    # e^{-C} normalization is folded into the per-step DVE
                    # copy-back instead of an ACT bias.
                    nc.scalar.activation(
                        t[:], t_raw[:], mybir.ActivationFunctionType.Exp
                    )
                    tiles.append(t)
                for j in range(nst):
                    ph = step % 2
                    ph2 = (step + 1) % 2
                    for g in range(2):
                        ps = psums[g]
                        for qg in range(2):
                            q = 2 * g + qg
                            lhsT = tiles[q][:, j * T : (j + 1) * T]
                            c_r = ph * 8 + 2 * q
                            c_w = ph2 * 4 + 2 * qg
                            nc.tensor.matmul(
                                ps[0:64, c_w : c_w + 1],
                                lhsT,
                                rhs[:, c_r : c_r + 1],
                                start=True,
                                stop=True,
                            )
                            nc.tensor.matmul(
                                ps[64:128, c_w + 1 : c_w + 2],
                                lhsT,
                                rhs[:, c_r + 1 : c_r + 2],
                                start=True,
                                stop=True,
                            )
                        nc.vector.tensor_scalar_mul(
                            rhs[:, ph2 * 8 + 4 * g : ph2 * 8 + 4 * g + 4],
                            ps[:, ph2 * 4 : ph2 * 4 + 4],
                            float(np.exp(-C_SHIFT)),
                        )
                    step += 1

            # export the final *unscaled* fp32 accumulator (one e^{-C} is
            # still owed; the host applies it in log space)
            parity = NSTEP % 2
            for g in range(2):
                nc.vector.tensor_copy(
                    out_stage[:, 4 * g : 4 * g + 4],
                    psums[g][:, parity * 4 : parity * 4 + 4],
                )
            nc.sync.dma_start(out_d[:], out_stage[:])
    nc.compile()
    return nc


def _get_nc():
    with _nc_lock:
        if _nc_cache[0] is None:
            _nc_cache[0] = _build_nc()
        return _nc_cache[0]


def _ensure_axon_hooks():
    """Provide antenv.axon_hooks (missing in this image) so that
    run_bass_kernel_spmd(trace=True) can register the NTFF profile hook."""
    import sys
    import types

    try:
        import antenv.axon_hooks  # noqa: F401
        return
    except ImportError:
        pass
    import antenv

    mod = types.ModuleType("antenv.axon_hooks")
    _hook = [None]
    mod.set_axon_ntff_profile_hook = lambda h: _hook.__setitem__(0, h)
    mod.get_axon_ntff_profile_hook = lambda: _hook[0]
    sys.modules["antenv.axon_hooks"] = mod
    antenv.axon_hooks = mod
    try:
        from trn_agent_boot.trn_boot import _ntff_profile_via_ctypes

        h = _ntff_profile_via_ctypes("/opt/axon/libaxon_pjrt.so")
        if h is not None:
            mod.set_axon_ntff_profile_hook(h)
    except Exception:
        pass


def kernel(scores, target, mask, antor_score, aid, **_unused):
    from concourse.bass_utils import run_bass_kernel_spmd

    scores = np.asarray(scores, dtype=np.float32)
    target = np.asarray(target)
    mask = np.asarray(mask)
    antor_score = np.asarray(antor_score, dtype=np.float32)
    aid = int(np.asarray(aid))
    assert scores.shape == (L, B, T, T), scores.shape

    mask_all = bool(mask.all())

    # ---- host prep: shard batch, build initial vectors ----
    p0 = scores[0, :, START_TAG, :].astype(np.float64)          # (B, T)
    s0 = p0.max(axis=1)                                          # (B,)
    w0 = np.exp(p0 - s0[:, None]).astype(np.float32)             # (B, T)

    def make_shard(c):
        sh = np.ascontiguousarray(scores[:, c * B_LOC : (c + 1) * B_LOC])
        if not mask_all:
            # a masked step must leave the partition unchanged:
            # E = e^{-C} * I  <=>  scores_eff = 0 on diag, -inf off-diag
            mloc = mask[:, c * B_LOC : (c + 1) * B_LOC]
            eye = np.full((T, T), -1e30, dtype=np.float32)
            np.fill_diagonal(eye, 0.0)
            ls, lb = np.nonzero(~mloc)
            sh[ls, lb] = eye
        return sh

    shards = [None] * NCORES
    threads = [
        threading.Thread(target=lambda c=c: shards.__setitem__(c, make_shard(c)))
        for c in range(NCORES)
    ]
    for t in threads:
        t.start()
    for t in threads:
        t.join()

    in_maps = []
    for c in range(NCORES):
        rhs_init = np.zeros((128, 16), dtype=np.float32)
        for b in range(B_LOC):
            q, half = b // 2, b % 2
            col = 2 * q + half
            rhs_init[half * 64 : half * 64 + 64, col] = w0[c * B_LOC + b]
        in_maps.append({"scores_loc": shards[c], "rhs_init": rhs_init})

    nc = _get_nc()
    do_trace = bool(int(os.environ.get("KERNEL_TRACE", "0")))
    if do_trace:
        _ensure_axon_hooks()
    try:
        res = run_bass_kernel_spmd(nc, in_maps, list(range(NCORES)), trace=do_trace)
    except Exception:
        if not do_trace:
            raise
        res = run_bass_kernel_spmd(nc, in_maps, list(range(NCORES)), trace=False)
    LAST_RESULTS[0] = res

    # ---- host finish ----
    # w_out holds the final step's *unscaled* accumulator: one e^{-C} is
    # still owed, i.e. partition = log(acc) - C + s0 + NSTEP*C
    Z = 0.0
    for c in range(NCORES):
        out = res.results[c]["w_out"]
        for b in range(B_LOC):
            q, half = b // 2, b % 2
            acc_end = float(out[half * 64 + END_TAG, 2 * q + half])
            Z += np.log(acc_end) + s0[c * B_LOC + b] + (NSTEP - 1) * C_SHIFT

    maskf = mask.astype(np.float64)
    tg = np.take_along_axis(
        scores.reshape(L, B, T * T), np.asarray(target, np.int64)[:, :, None], axis=2
    )[..., 0]
    tg_energy = float((tg * maskf).sum())

    a = antor_score.astype(np.float64)
    wsm = np.exp(a - a.max())
    wsm /= wsm.sum()
    loss = (Z - tg_energy) * wsm[aid] / B
    return np.float32(loss)



# revision 4
# speedup vs baseline: 1.3154x; 1.3154x over previous
"""CRF loss (forward-algorithm partition function) on 8 Trainium2 cores.

Strategy (v2 — segment-parallel matrix chain)
---------------------------------------------
Batch (B=64) is sharded 8 ways -> 8 sequences per core.  The log-space scan
is computed in *linear* space:  with  E_l = exp(scores_l - C),
C = log(T) + 0.5, the recurrence becomes  w_l = E_l^T w_{l-1}.

Instead of a 511-step sequential vector chain (latency-bound: each tiny
matvec waits on the previous step's PSUM->SBUF copy), each chain is split
into S=4 *segments* of 128 matrices (one identity pad at the global front).
Each segment reduces independently via matrix-matrix products
A_j = E_j^T A_{j-1}  (A_0 = I), giving 32 independent streams per core
-> the TensorE pipeline stays full and no step waits on any other stream.
The host combines the 4 segment matrices per chain in float64 (trivial
flops) and applies gold-path energy / softmax weighting.

Per-core engine budget (cost model):
  - DMA in:  64 MB fp32 at full 360 GB/s  (host pre-packs the score image so
    every DMA descriptor moves 8 KB contiguous per partition; the baseline's
    256 B descriptors paid a 2x read-modify-write penalty)     ~186 us
  - ScalarE: exp(x - C) on [128, 2048] tiles + 1/4 of copies   ~167 us
  - TensorE: 4096 independent [64x64]@[64x64] bf16 matmuls     ~110 us
  - VectorE: 3/4 of the PSUM->SBUF bf16 accumulator copies     ~150 us
Streams are packed 2-per-tile (chains 2a/2a+1 at partitions 0:64/64:128);
matmul operands at partition offset 64 verified in CoreSim.
"""

import os
import threading
import numpy as np

L, B, T = 512, 64, 64
NCORES = 8
B_LOC = B // NCORES            # 8 sequences per core
NSEG = 4                       # segments per chain
NSTEP = 128                    # matrices per segment (incl. 1 identity pad)
NPAIR = 16                     # stream pairs per core: (s, a), a = chain pair
NBLK = 4                       # DMA/exp blocks per pair
WBLK = 32                      # steps per block (NSTEP = NBLK * WBLK)
C_SHIFT = float(np.log(T) + 0.5)
START_TAG = 0
END_TAG = 1
NEG = -1e30                    # "minus infinity" for identity-pad off-diagonals

_nc_cache = [None]
_nc_lock = threading.Lock()
LAST_RESULTS = [None]          # test.py reads exec_time_ns from here


def _build_nc():
    import concourse.bacc as bacc
    import concourse.mybir as mybir
    import concourse.tile as tile

    dt = mybir.dt
    nc = bacc.Bacc("TRN2", target_bir_lowering=False, debug=False)

    # [pair, block, partition, (w, u)] -- 8KB contiguous per partition per DMA
    img_d = nc.declare_dram_parameter(
        "img", [NPAIR, NBLK, 128, WBLK * T], dt.float32, isOutput=False
    )
    eye_d = nc.declare_dram_parameter("eye2", [128, T], dt.float32, isOutput=False)
    out_d = nc.declare_dram_parameter(
        "m_out", [128, NSEG * 4 * T], dt.float32, isOutput=True
    )

    with tile.TileContext(nc) as tc:
        with (
            tc.tile_pool(name="raw", bufs=3) as raw_pool,
            tc.tile_pool(name="exp", bufs=2) as exp_pool,
            tc.tile_pool(name="state", bufs=1) as state_pool,
            tc.tile_pool(name="psum", bufs=1, space="PSUM") as psum_pool,
        ):
            eye_stage = state_pool.tile([128, T], dt.float32)
            eye_bf = state_pool.tile([128, T], dt.bfloat16)
            bias_c = state_pool.tile([128, 1], dt.float32)
            nc.gpsimd.memset(bias_c[:], -C_SHIFT)
            out_stage = state_pool.tile([128, NSEG * 4 * T], dt.float32)
            # accumulators: one [128, 256] tile per (segment-group, parity);
            # pair a of group g lives at cols a*64, chain 2a at partitions
            # 0:64, chain 2a+1 at 64:128.
            acc = [
                [
                    state_pool.tile([128, 4 * T], dt.bfloat16, name=f"acc_g{g}p{p}")
                    for p in range(2)
                ]
                for g in range(NSEG)
            ]
            # PSUM: parity-0 tiles first then parity-1 so a group's two
            # parities land in different banks (PE-W vs DVE-R same-bank
            # accesses serialize in hardware).
            psum_tiles = {}
            for p in range(2):
                for g in range(NSEG):
                    psum_tiles[(g, p)] = psum_pool.tile(
                        [128, 4 * T], dt.float32, name=f"ps_g{g}p{p}"
                    )

            nc.sync.dma_start(eye_stage[:], eye_d[:])
            nc.vector.tensor_copy(eye_bf[:], eye_stage[:])
            for g in range(NSEG):
                for a in range(4):
                    nc.vector.tensor_copy(
                        acc[g][0][:, a * T : (a + 1) * T], eye_bf[:]
                    )

            # ---- stream in all score blocks (SP queue drains as raw bufs
            # free up; order (k, q) matches consumption order) ----
            raw_tiles = {}
            for k in range(NBLK):
                for q in range(NPAIR):
                    t_raw = raw_pool.tile([128, WBLK * T], dt.float32, tag="raw")
                    nc.sync.dma_start(t_raw[:], img_d[q, k])
                    raw_tiles[(q, k)] = t_raw

            # ---- exp for block 0 of every pair ----
            exp_tiles = {}

            def emit_exp(q, k):
                t = exp_pool.tile([128, WBLK * T], dt.bfloat16, tag=f"exp{q}")
                nc.scalar.activation(
                    t[:],
                    raw_tiles[(q, k)][:],
                    mybir.ActivationFunctionType.Exp,
                    bias=bias_c[:, 0:1],
                )
                exp_tiles[(q, k)] = t

            for q in range(NPAIR):
                emit_exp(q, 0)

            # ---- 128 lockstep rounds over 32 independent streams ----
            for r in range(NSTEP):
                kblk, w = r // WBLK, r % WBLK
                # spread next block's 16 exps over this block's 32 rounds
                if kblk < NBLK - 1 and r % 2 == 0:
                    emit_exp((r % WBLK) // 2, kblk + 1)
                rp = r % 2
                last = r == NSTEP - 1
                for g in range(NSEG):
                    ps = psum_tiles[(g, rp)]
                    a_in = acc[g][rp]
                    for a in range(4):
                        q = g * 4 + a
                        et = exp_tiles[(q, kblk)]
                        lo = slice(0, T)
                        hi = slice(T, 2 * T)
                        cw = slice(w * T, (w + 1) * T)
                        ca = slice(a * T, (a + 1) * T)
                        nc.tensor.matmul(
                            ps[lo, ca], et[lo, cw], a_in[lo, ca],
                            start=True, stop=True,
                        )
                        nc.tensor.matmul(
                            ps[hi, ca], et[hi, cw], a_in[hi, ca],
                            start=True, stop=True,
                        )
                    # copy this group's 8 products back (bf16 for the next
                    # round; fp32 to the staging tile on the last round).
                    # groups 0-2 on VectorE, group 3 on ScalarE.
                    if last:
                        dst = out_stage[:, g * 4 * T : (g + 1) * 4 * T]
                    else:
                        dst = acc[g][(r + 1) % 2][:]
                    if g == NSEG - 1:
                        nc.scalar.activation(
                            dst, ps[:], mybir.ActivationFunctionType.Copy
                        )
                    else:
                        nc.vector.tensor_copy(dst, ps[:])

            nc.sync.dma_start(out_d[:], out_stage[:])
    nc.compile()
    return nc


def _get_nc():
    with _nc_lock:
        if _nc_cache[0] is None:
            _nc_cache[0] = _build_nc()
        return _nc_cache[0]


def _ensure_axon_hooks():
    """Provide antenv.axon_hooks (missing in this image) so that
    run_bass_kernel_spmd(trace=True) can register the NTFF profile hook."""
    import sys
    import types

    try:
        import antenv.axon_hooks  # noqa: F401
        return
    except ImportError:
        pass
    import antenv

    mod = types.ModuleType("antenv.axon_hooks")
    _hook = [None]
    mod.set_axon_ntff_profile_hook = lambda h: _hook.__setitem__(0, h)
    mod.get_axon_ntff_profile_hook = lambda: _hook[0]
    sys.modules["antenv.axon_hooks"] = mod
    antenv.axon_hooks = mod
    try:
        from trn_agent_boot.trn_boot import _ntff_profile_via_ctypes

        h = _ntff_profile_via_ctypes("/opt/axon/libaxon_pjrt.so")
        if h is not None:
            mod.set_axon_ntff_profile_hook(h)
    except Exception:
        pass


def _build_image(scores, mask, mask_all, c):
    """Per-core DMA image [NPAIR, NBLK, 128, WBLK*T] fp32.

    img[(s,a), k, (h,t), (w,u)] = padded[s*128 + k*32 + w, (a,h), t, u]
    where padded[0] is the identity-pad matrix (diag +C, off-diag -1e30 so
    exp(x - C) == I exactly) and padded[m] = scores[m] for m >= 1.
    """
    sh = scores[:, c * B_LOC : (c + 1) * B_LOC]  # (512, 8, 64, 64) view
    padded = np.empty((L, B_LOC, T, T), dtype=np.float32)
    padded[1:] = sh[1:]
    pad = np.full((T, T), NEG, dtype=np.float32)
    np.fill_diagonal(pad, C_SHIFT)
    padded[0] = pad
    if not mask_all:
        # a masked step must leave the partition unchanged: exp(x - C) = I
        mloc = mask[:, c * B_LOC : (c + 1) * B_LOC]
        ls, lb = np.nonzero(~mloc)
        for li, bi in zip(ls, lb):
            if li >= 1:
                padded[li, bi] = pad
    # (s,k,w, a,h, t, u) <- (m=(s,k,w), c=(a,h), t, u)
    v = padded.reshape(NSEG, NBLK, WBLK, 4, 2, T, T)
    img = np.ascontiguousarray(v.transpose(0, 3, 1, 4, 5, 2, 6)).reshape(
        NPAIR, NBLK, 128, WBLK * T
    )
    return img


def kernel(scores, target, mask, antor_score, aid, **_unused):
    from concourse.bass_utils import run_bass_kernel_spmd

    scores = np.asarray(scores, dtype=np.float32)
    target = np.asarray(target)
    mask = np.asarray(mask)
    antor_score = np.asarray(antor_score, dtype=np.float32)
    aid = int(np.asarray(aid))
    assert scores.shape == (L, B, T, T), scores.shape

    mask_all = bool(mask.all())

    # ---- host prep: initial vectors + per-core DMA images ----
    p0 = scores[0, :, START_TAG, :].astype(np.float64)          # (B, T)
    s0 = p0.max(axis=1)                                          # (B,)
    w0 = np.exp(p0 - s0[:, None])                                # (B, T) f64

    eye2 = np.tile(np.eye(T, dtype=np.float32), (2, 1))          # (128, 64)

    imgs = [None] * NCORES
    threads = [
        threading.Thread(
            target=lambda c=c: imgs.__setitem__(
                c, _build_image(scores, mask, mask_all, c)
            )
        )
        for c in range(NCORES)
    ]
    for t in threads:
        t.start()
    for t in threads:
        t.join()

    in_maps = [{"img": imgs[c], "eye2": eye2} for c in range(NCORES)]

    nc = _get_nc()
    do_trace = bool(int(os.environ.get("KERNEL_TRACE", "0")))
    if do_trace:
        _ensure_axon_hooks()
    try:
        res = run_bass_kernel_spmd(nc, in_maps, list(range(NCORES)), trace=do_trace)
    except Exception:
        if not do_trace:
            raise
        res = run_bass_kernel_spmd(nc, in_maps, list(range(NCORES)), trace=False)
    LAST_RESULTS[0] = res

    # ---- host combine (float64): Z_b = log(w[END]) + renorms + 511*C + s0 ----
    # m_out[(h,t'), s*256 + a*64 + n] = M_{chain 2a+h, seg s}[t', n]
    Z = 0.0
    for c in range(NCORES):
        out = np.asarray(res.results[c]["m_out"], dtype=np.float64)
        for bl in range(B_LOC):
            a, h = bl // 2, bl % 2
            b = c * B_LOC + bl
            w = w0[b].copy()
            logacc = 0.0
            for s in range(NSEG):
                M = out[h * T : (h + 1) * T, s * 4 * T + a * T : s * 4 * T + (a + 1) * T]
                w = M @ w
                mx = w.max()
                w /= mx
                logacc += np.log(mx)
            Z += np.log(w[END_TAG]) + logacc + s0[b] + (L - 1) * C_SHIFT

    maskf = mask.astype(np.float64)
    tg = np.take_along_axis(
        scores.reshape(L, B, T * T), np.asarray(target, np.int64)[:, :, None], axis=2
    )[..., 0]
    tg_energy = float((tg * maskf).sum())

    a = antor_score.astype(np.float64)
    wsm = np.exp(a - a.max())
    wsm /= wsm.sum()
    loss = (Z - tg_energy) * wsm[aid] / B
    return np.float32(loss)
